# revision 1
# baseline (speedup 1.0000x reference)
#!/usr/bin/env python
"""Multi-head attention (nn_MultiHeadAttention) Trainium2 Bass kernel.

Problem: B=8, S=1024, n_hidden=1024, 16 heads x 64 dim. V projection == K
projection (reference quirk). Output = softmax(mask + QK^T/8) @ K @ Wo + bo.

Strategy: batch-parallel across the 8 NeuronCores (core b handles batch b,
weights replicated, zero collectives). Per core, everything is computed in
"transposed" (feature-on-partition) layout so the tensor engine contracts
along partitions naturally:

  x^T  [hid, s]   via PE transposes of x
  Q^T = Wq^T x^T, K^T = Wk^T x^T         (fp32r matmuls, 1 cycle/row)
  V_h  [s, dh]    via PE transposes of K^T_h, augmented with a ones column
  logits^T_h [k, q] = (K^T_h)^T-contract Q^T_h   (K=64; head pairs run
                      concurrently in disjoint PE row groups)
  E_h = exp(logits^T/8 + mask_bias)      (ACT, per-partition bias = key mask)
  att^T_h [dh+1, q] = V_h^T-contract E_h (row dh = softmax denominator)
  normalize via PE-broadcast reciprocal + DVE multiply
  out [q, m] = att^T-contract Wo + bo    (direct DRAM layout)

The softmax skips the max-subtraction: logits are O(6), exp stays in fp32
range, masked keys produce exp(-1e9) == 0 exactly.

All matmul operands are float32r (fp32 storage, reduced-precision PE mode,
4x faster than fp32). Non-matmul math stays fp32.
"""
import sys
import os

sys.path.insert(0, "/opt/trn_rl_repo")
os.environ.setdefault("JAX_COMPILATION_CACHE_DIR", "/tmp/jax_comp_cache")

import numpy as np

B, S, H, NH, DH = 8, 1024, 1024, 16, 64
NT = H // 128   # 8 partition tiles of hidden
NCH = S // 128  # 8 key chunks
NQ = S // 512   # 2 query 512-tiles

_cache = {}

_PHASES = os.environ.get("KERNEL_PHASES", "ABDTE")


def _build_nc():
    import concourse.bacc as bacc
    import concourse.tile as tile
    from concourse import mybir
    from contextlib import ExitStack

    F32 = mybir.dt.float32
    F32R = mybir.dt.float32r
    AF = mybir.ActivationFunctionType

    nc = bacc.Bacc("TRN2", target_bir_lowering=False, debug=False)

    x_d = nc.dram_tensor("x", [S, H], F32R, kind="ExternalInput").ap()
    maskf_d = nc.dram_tensor("maskf", [128, NCH], F32, kind="ExternalInput").ap()
    wq_d = nc.dram_tensor("wq", [H, H], F32R, kind="ExternalInput").ap()  # pre-tiled [m*128+p, k*128+mm]
    wk_d = nc.dram_tensor("wk", [H, H], F32R, kind="ExternalInput").ap()  # pre-tiled
    wo_d = nc.dram_tensor("wo", [H, H], F32R, kind="ExternalInput").ap()
    bqr_d = nc.dram_tensor("bqr", [128, NT], F32, kind="ExternalInput").ap()
    bkr_d = nc.dram_tensor("bkr", [128, NT], F32, kind="ExternalInput").ap()
    bo_d = nc.dram_tensor("bo_bc", [128, H], F32, kind="ExternalInput").ap()
    id_d = nc.dram_tensor("ident", [128, 128], F32R, kind="ExternalInput").ap()
    sel_d = nc.dram_tensor("sel", [4, 2 * 128], F32R, kind="ExternalInput").ap()
    vones_d = nc.dram_tensor("vones", [128, NH * NCH], F32R, kind="ExternalInput").ap()
    out_d = nc.dram_tensor("out", [S, H], F32, kind="ExternalOutput").ap()
    dscr_d = nc.dram_tensor("dscr", [NH, S], F32).ap()  # denominator bounce buffer

    with tile.TileContext(nc) as tc, ExitStack() as top:
        misc = top.enter_context(tc.tile_pool(name="misc", bufs=1))
        maskf = misc.tile([128, NCH], F32)
        bqr = misc.tile([128, NT], F32)
        bkr = misc.tile([128, NT], F32)
        bo_bc = misc.tile([128, H], F32)
        ident = misc.tile([128, 128], F32R)
        sel = misc.tile([4, 2 * 128], F32R)
        denomA = misc.tile([4, 4 * S], F32)
        recipA = misc.tile([4, 4 * S], F32R)

        # One [128, 8K] slot reused: xT during phases A..C, attT during D..E.
        big_p = top.enter_context(tc.tile_pool(name="big", bufs=1))

        # ---- Phase A: load x, transpose to x^T -------------------------------
        with tc.tile_pool(name="xs", bufs=1) as xs_p, \
             tc.tile_pool(name="tp", bufs=4, space="PSUM") as tp_p:
            xT = big_p.tile([128, NT * S], F32R, tag="big", name="xT")
            xs = xs_p.tile([128, NCH * H], F32R)
            for sc in range(NCH):
                nc.sync.dma_start(
                    xs[:, sc * H : (sc + 1) * H],
                    x_d[sc * 128 : (sc + 1) * 128, :],
                )

            nc.sync.dma_start(ident[:], id_d)
            nc.sync.dma_start(maskf[:], maskf_d)
            nc.sync.dma_start(bqr[:], bqr_d)
            nc.sync.dma_start(bkr[:], bkr_d)
            nc.sync.dma_start(bo_bc[:], bo_d)
            nc.sync.dma_start(sel[:], sel_d)
            for g in range(2):
                for hc in range(NT):
                    pt = tp_p.tile([128, 512], F32R, tag="tp")
                    for j in range(4):
                        sc = g * 4 + j
                        nc.tensor.transpose(
                            pt[:, 128 * j : 128 * (j + 1)],
                            xs[:, sc * H + hc * 128 : sc * H + (hc + 1) * 128],
                            ident[:],
                        )
                    xt_dst = xT[:, hc * S + g * 512 : hc * S + (g + 1) * 512]
                    if hc % 2 == 0:
                        nc.vector.tensor_copy(xt_dst, pt[:])
                    else:
                        nc.scalar.activation(xt_dst, pt[:], AF.Identity, bias=0.0)

        qkv = ExitStack()
        QT_p = qkv.enter_context(tc.tile_pool(name="QT", bufs=1))
        KT_p = qkv.enter_context(tc.tile_pool(name="KT", bufs=1))
        V_p = qkv.enter_context(tc.tile_pool(name="V", bufs=1))
        QT = QT_p.tile([128, NT * S], F32R)
        KT = KT_p.tile([128, NT * S], F32R)
        V = V_p.tile([128, NH * NCH * (DH + 1)], F32R)
        V_blocks = V[:].rearrange("p (g o) -> p g o", o=DH + 1)
        nc.sync.dma_start(
            V_blocks[:, :, DH : DH + 1],
            vones_d.rearrange("p (g o) -> p g o", o=1),
        )

        # ---- Phase B+C: projections and V transposes -------------------------
        with tc.tile_pool(name="wst", bufs=10) as wst_p, \
             tc.tile_pool(name="proj", bufs=8, space="PSUM") as proj_p:
            def _w_dma(w_d, m, nm):
                w_m = wst_p.tile([128, NT * 128], F32R, tag="w", name=nm)
                nc.sync.dma_start(w_m[:], w_d[m * 128 : (m + 1) * 128, :])
                return w_m

            nb = NT if "B" in _PHASES else 0
            pend = {}
            for pm in range(min(4, nb)):
                pend[pm] = (_w_dma(wq_d, pm, f"wq_{pm}"), _w_dma(wk_d, pm, f"wk_{pm}"))
            for m in range(nb):
                wq_m, wk_m = pend.pop(m)
                if m + 4 < nb:
                    pend[m + 4] = (
                        _w_dma(wq_d, m + 4, f"wq_{m+4}"),
                        _w_dma(wk_d, m + 4, f"wk_{m+4}"),
                    )
                for dst, w_m, brow, on_act in (
                    (QT, wq_m, bqr, True),
                    (KT, wk_m, bkr, False),
                ):
                    for n in range(NQ):
                        pp = proj_p.tile([128, 512], F32, tag="pj")
                        for k in range(NT):
                            nc.tensor.matmul(
                                pp[:],
                                w_m[:, k * 128 : (k + 1) * 128],
                                xT[:, k * S + n * 512 : k * S + (n + 1) * 512],
                                start=(k == 0),
                                stop=(k == NT - 1),
                            )
                        osl = dst[:, m * S + n * 512 : m * S + (n + 1) * 512]
                        if on_act:
                            nc.scalar.activation(
                                osl, pp[:], AF.Identity, bias=brow[:, m : m + 1]
                            )
                        else:
                            nc.vector.tensor_scalar_add(osl, pp[:], brow[:, m : m + 1])
                # V transposes for the two heads in tile m
                for h2 in (0, 1):
                    h = 2 * m + h2
                    pv = proj_p.tile([128, 512], F32R, tag="pj")
                    for c in range(NCH):
                        nc.tensor.transpose(
                            pv[:, c * DH : (c + 1) * DH],
                            KT[64 * h2 : 64 * h2 + 64, m * S + c * 128 : m * S + (c + 1) * 128],
                            ident[64 * h2 : 64 * h2 + 64, 64 * h2 : 64 * h2 + 64],
                        )
                    nc.vector.tensor_copy(
                        V_blocks[:, h * NCH : (h + 1) * NCH, 0:DH],
                        pv[:].rearrange("p (c d) -> p c d", d=DH),
                    )

        # ---- Phase D: attention, head pair per hidden tile -------------------
        with tc.tile_pool(name="lg", bufs=2, space="PSUM") as lg_p, \
             tc.tile_pool(name="attp", bufs=2, space="PSUM") as att_p, \
             tc.tile_pool(name="E", bufs=6) as E_p, \
             tc.tile_pool(name="st", bufs=3) as st_p:
            attT = big_p.tile([128, NT * S], F32R, tag="big", name="attT")
            for t in range(NT if "D" in _PHASES else 0):
                aps = [att_p.tile([128, S], F32, tag="att", name=f"att_{t}_{i}") for i in (0, 1)]
                for c in range(NCH):
                    # adjacent base-0 / base-64 logits MMs -> PE row-group overlap
                    lgs = []
                    for h2 in (0, 1):
                        lg = lg_p.tile([128, S], F32, tag="lg", name=f"lg_{t}_{c}_{h2}")
                        for n in range(NQ):
                            nc.tensor.matmul(
                                lg[:, n * 512 : (n + 1) * 512],
                                KT[64 * h2 : 64 * h2 + 64, t * S + c * 128 : t * S + (c + 1) * 128],
                                QT[64 * h2 : 64 * h2 + 64, t * S + n * 512 : t * S + (n + 1) * 512],
                                start=True,
                                stop=True,
                            )
                        lgs.append(lg)
                    Es = []
                    for h2 in (0, 1):
                        E_t = E_p.tile([128, S], F32R, tag="E", name=f"E_{t}_{c}_{h2}")
                        nc.scalar.activation(
                            E_t[:], lgs[h2][:], AF.Exp, bias=maskf[:, c : c + 1], scale=0.125
                        )
                        Es.append(E_t)
                    for h2 in (0, 1):
                        h = 2 * t + h2
                        for n in range(NQ):
                            nc.tensor.matmul(
                                aps[h2][0 : DH + 1, n * 512 : (n + 1) * 512],
                                V[:, (h * NCH + c) * (DH + 1) : (h * NCH + c + 1) * (DH + 1)],
                                Es[h2][:, n * 512 : (n + 1) * 512],
                                start=(c == 0),
                                stop=(c == NCH - 1),
                            )
                for h2 in (0, 1):
                    h = 2 * t + h2
                    st = st_p.tile([128, S], F32, tag="st")
                    nc.vector.tensor_copy(st[64:65, :], aps[h2][DH : DH + 1, :])
                    nc.sync.dma_start(dscr_d[h : h + 1, :], st[64:65, :])
                    nc.vector.tensor_copy(
                        attT[64 * h2 : 64 * h2 + 64, t * S : (t + 1) * S],
                        aps[h2][0:DH, :],
                    )
                if t % 2 == 1:
                    u = t // 2
                    nc.sync.dma_start(
                        denomA[0:4, u * S : (u + 1) * S].rearrange(
                            "p (a q) -> p a q", a=1
                        ),
                        dscr_d[4 * u : 4 * (u + 1), :].rearrange(
                            "(p a) q -> p a q", a=1
                        ),
                    )
                    with nc.allow_low_precision(reason="softmax reciprocal in f32r"):
                        nc.vector.reciprocal(
                            recipA[0:4, u * S : (u + 1) * S],
                            denomA[0:4, u * S : (u + 1) * S],
                        )

            # ---- Phase D tail: broadcast reciprocals, normalize (reuses lg slots)
            for t in range(NT if "T" in _PHASES else 0):
                rbc = lg_p.tile([128, S], F32, tag="lg", name=f"rbc_{t}")
                u, par = t // 2, t % 2
                for n in range(NQ):
                    nc.tensor.matmul(
                        rbc[:, n * 512 : (n + 1) * 512],
                        sel[:, par * 128 : (par + 1) * 128],
                        recipA[0:4, u * S + n * 512 : u * S + (n + 1) * 512],
                        start=True,
                        stop=True,
                    )
                for n in range(NQ):
                    sl = slice(t * S + n * 512, t * S + (n + 1) * 512)
                    nc.vector.tensor_mul(attT[:, sl], attT[:, sl], rbc[:, n * 512 : (n + 1) * 512])

        qkv.close()

        # ---- Phase E prologue: prefetch Wo while the tail runs ---------------
        wo_cm = tc.tile_pool(name="wo", bufs=1)
        wo_p = wo_cm.__enter__()
        wo_sb = wo_p.tile([128, NT * H], F32R)
        if "E" in _PHASES:
            for c in range(NT):
                nc.sync.dma_start(
                    wo_sb[:, c * H : (c + 1) * H],
                    wo_d[c * 128 : (c + 1) * 128, :],
                )

        # ---- Phase E: output projection --------------------------------------
        with tc.tile_pool(name="op", bufs=4, space="PSUM") as op_p, \
             tc.tile_pool(name="os", bufs=3) as os_p:
            for qt in range(NT if "E" in _PHASES else 0):
                for mt in range(NQ):
                    po = op_p.tile([128, 512], F32, tag="op")
                    for c in range(NT):
                        nc.tensor.matmul(
                            po[:],
                            attT[:, c * S + qt * 128 : c * S + (qt + 1) * 128],
                            wo_sb[:, c * H + mt * 512 : c * H + (mt + 1) * 512],
                            start=(c == 0),
                            stop=(c == NT - 1),
                        )
                    ob = os_p.tile([128, 512], F32, tag="os")
                    nc.vector.tensor_add(ob[:], po[:], bo_bc[:, mt * 512 : (mt + 1) * 512])
                    nc.sync.dma_start(
                        out_d[qt * 128 : (qt + 1) * 128, mt * 512 : (mt + 1) * 512], ob[:]
                    )
        wo_cm.__exit__(None, None, None)

    nc.compile()
    return nc


def _host_inputs(inputs):
    """Host-side prep: per-core input dicts (core b <- batch b)."""
    x = np.asarray(inputs["x"], dtype=np.float32)
    mask = np.asarray(inputs["padding_mask"])
    def _pretile(w):
        # w[k*128+p, m*128+mm] -> out[m*128+p, k*128+mm]
        w = np.asarray(w, dtype=np.float32).reshape(NT, 128, NT, 128)
        return np.ascontiguousarray(w.transpose(2, 1, 0, 3).reshape(H, H))

    wq = _pretile(inputs["Wq"])
    wk = _pretile(inputs["Wk"])
    wo = np.ascontiguousarray(np.asarray(inputs["Wo"], dtype=np.float32))
    bq = np.asarray(inputs["bq"], dtype=np.float32)
    bk = np.asarray(inputs["bk"], dtype=np.float32)
    bo = np.asarray(inputs["bo"], dtype=np.float32)

    bqr = np.ascontiguousarray(bq.reshape(NT, 128).T)
    bkr = np.ascontiguousarray(bk.reshape(NT, 128).T)
    bo_bc = np.ascontiguousarray(np.tile(bo[None, :], (128, 1)))
    ident = np.eye(128, dtype=np.float32)
    sel = np.zeros((4, 2 * 128), dtype=np.float32)
    for par in range(2):
        for p in range(128):
            sel[2 * par + p // 64, par * 128 + p] = 1.0

    in_maps = []
    for b in range(B):
        maskf = np.ascontiguousarray(
            mask[b].astype(np.float32).reshape(NCH, 128).T * -1.0e9
        )
        in_maps.append(
            {
                "x": np.ascontiguousarray(x[b]),
                "maskf": maskf,
                "wq": wq,
                "wk": wk,
                "wo": wo,
                "bqr": bqr,
                "bkr": bkr,
                "bo_bc": bo_bc,
                "ident": ident,
                "sel": sel,
                "vones": np.ones((128, NH * NCH), dtype=np.float32),
            }
        )
    return in_maps


def _get_nc():
    if "nc" not in _cache:
        _cache["nc"] = _build_nc()
    return _cache["nc"]


def kernel(**inputs):
    from concourse.bass_utils import run_bass_kernel_spmd

    nc = _get_nc()
    in_maps = _host_inputs(inputs)
    res = run_bass_kernel_spmd(nc, in_maps, core_ids=list(range(B)))
    out = np.stack([res.results[b]["out"] for b in range(B)], axis=0)
    return out.astype(np.float32, copy=False)


def _get_runner():
    """Cached jitted SPMD executable (mirrors bass2jax.run_bass_via_pjrt) so
    repeat executions skip retrace/recompile — used for timing."""
    if "runner" in _cache:
        return _cache["runner"]
    import jax
    import jax.numpy as jnp
    from jax.sharding import Mesh, PartitionSpec
    from jax.experimental.shard_map import shard_map
    from concourse import mybir
    from concourse import bass2jax

    nc = _get_nc()
    bass2jax.install_neuronx_cc_hook()
    part_name = nc.partition_id_tensor.name if nc.partition_id_tensor else None
    in_names, out_names, out_avals, zero_outs = [], [], [], []
    for alloc in nc.m.functions[0].allocations:
        if not isinstance(alloc, mybir.MemoryLocationSet):
            continue
        name = alloc.memorylocations[0].name
        if alloc.kind == "ExternalInput":
            if name != part_name:
                in_names.append(name)
        elif alloc.kind == "ExternalOutput":
            out_names.append(name)
            shape = tuple(alloc.tensor_shape)
            dtype = mybir.dt.np(alloc.dtype)
            out_avals.append(jax.core.ShapedArray(shape, dtype))
            zero_outs.append(np.zeros(shape, dtype))
    n_params = len(in_names)
    all_in_names = in_names + out_names
    if part_name is not None:
        all_in_names = all_in_names + [part_name]

    def _body(*args):
        operands = list(args)
        if part_name is not None:
            operands.append(bass2jax.partition_id_tensor())
        outs = bass2jax._bass_exec_p.bind(
            *operands,
            out_avals=tuple(out_avals),
            in_names=tuple(all_in_names),
            out_names=tuple(out_names),
            lowering_input_output_aliases=(),
            sim_require_finite=True,
            sim_require_nnan=True,
            nc=nc,
        )
        return tuple(outs)

    devices = jax.devices()[:B]
    mesh = Mesh(np.asarray(devices), ("core",))
    n_outs = len(out_names)
    sharded = jax.jit(
        shard_map(
            _body,
            mesh=mesh,
            in_specs=(PartitionSpec("core"),) * (n_params + n_outs),
            out_specs=(PartitionSpec("core"),) * n_outs,
            check_rep=False,
        ),
        keep_unused=True,
    )
    _cache["runner"] = (sharded, in_names, out_names, zero_outs, mesh)
    return _cache["runner"]


def timed_run(inputs, n_iter=8):
    """Dispatch the kernel n_iter times back-to-back and return the amortized
    per-execution wall time in ns (pipelined dispatch hides host/tunnel
    latency; device execution serializes per-device)."""
    import jax, time
    from jax.sharding import NamedSharding, PartitionSpec

    sharded, in_names, out_names, zero_outs, mesh = _get_runner()
    in_maps = _host_inputs(inputs)
    concat_in = [
        np.concatenate([np.asarray(in_maps[c][n]) for c in range(B)], axis=0)
        for n in in_names
    ]
    concat_zeros = [
        np.zeros((B * z.shape[0], *z.shape[1:]), z.dtype) for z in zero_outs
    ]
    sh = NamedSharding(mesh, PartitionSpec("core"))
    args = [jax.device_put(a, sh) for a in concat_in + concat_zeros]
    jax.block_until_ready(args)
    # warm (compile + first exec)
    out = sharded(*args)
    jax.block_until_ready(out)
    t0 = time.time()
    outs = [sharded(*args) for _ in range(n_iter)]
    jax.block_until_ready(outs)
    dt = time.time() - t0
    return dt / n_iter * 1e9

